# revision 10
# baseline (speedup 1.0000x reference)
"""Trainium2 Bass kernel for nn_AMRNL (gnn_message_passing).

Data-parallel over batch on 8 NeuronCores: core i handles batch rows
[32*i, 32*i+32). Per core: 64 LSTM sequences (32 question + 32 answer),
two-level embedding gather (content ids -> tokens -> word2vec rows, with
the word2vec gather landing TRANSPOSED via HWDGE xbar DMA-transpose so the
input projection matmul needs no on-chip transposes), 100-step LSTM
recurrence with the input projection accumulated directly into the gates
PSUM chunks, bilinear+cosine scoring head, and degree-normalized neighbor
aggregation.

Self-contained: hardcodes all shapes; host-side prep is limited to weight
layout transforms (transpose/pad/gate-permute), index-list sharding, and
dtype casts.
"""

import sys

sys.path.insert(0, "/opt/trn_rl_repo")

import numpy as np
import ml_dtypes

# ---- problem dims (hardcoded) ----
B = 256
L = 100
E = 300
H = 256
V = 50000
U = 10000
NCONT = 20000
K = 32
SMOOTH = 1.0

N_CORES = 8
RPC = B // N_CORES          # rows per core = 32
S = 2 * RPC                 # sequences per core = 64
EP = 384                    # padded embedding dim (3 x 128)
G4 = 4 * H                  # 1024 gate width
NCHUNK = L // 2             # 50 psum chunks (2 steps each)
LOOKAHEAD = 3               # embedding-chunk prefetch depth

_CACHE = {}


def _build():
    import concourse.bass as bass
    import concourse.bacc as bacc
    import concourse.mybir as mybir
    import concourse.tile as tile
    sys.path.insert(0, "/root/problem")

    f32 = mybir.dt.float32
    f32r = mybir.dt.float32r
    bf16 = mybir.dt.bfloat16
    i32 = mybir.dt.int32

    nc = bacc.Bacc("TRN2")

    # ---- external inputs (per-core shards / replicated tables) ----
    cids = nc.declare_dram_parameter("cids", [S, 1], i32, isOutput=False)
    users = nc.declare_dram_parameter("users", [RPC, 1], i32, isOutput=False)
    w2v = nc.declare_dram_parameter("w2v", [V, EP], f32, isOutput=False)
    ctab = nc.declare_dram_parameter("ctab", [NCONT, L], i32, isOutput=False)
    wih_t = nc.declare_dram_parameter("wih_t", [EP, G4], bf16, isOutput=False)
    whh_t = nc.declare_dram_parameter("whh_t", [H, G4], f32r, isOutput=False)
    bil_w = nc.declare_dram_parameter("bil_w", [H, H], f32r, isOutput=False)
    bil_b = nc.declare_dram_parameter("bil_b", [RPC, 1], f32, isOutput=False)
    uemb = nc.declare_dram_parameter("uemb", [U + 2, H], f32, isOutput=False)
    adj = nc.declare_dram_parameter("adj", [U + 2, K], i32, isOutput=False)
    deg = nc.declare_dram_parameter("deg", [U + 2, 1], f32, isOutput=False)
    nbmask = nc.declare_dram_parameter("nbmask", [128, 8 * RPC], f32, isOutput=False)
    ident = nc.declare_dram_parameter("ident", [S, S], f32, isOutput=False)
    zeros = nc.declare_dram_parameter("zeros", [S, H], f32, isOutput=False)
    zerosr = nc.declare_dram_parameter("zerosr", [128, H], f32r, isOutput=False)

    out_res = nc.declare_dram_parameter("res", [RPC, 1], f32, isOutput=True)
    out_reg = nc.declare_dram_parameter("reg", [RPC, H], f32, isOutput=True)

    with tile.TileContext(nc) as tc:
        with tc.tile_pool(name="persist", bufs=1) as pp, \
             tc.tile_pool(name="dram", bufs=1, space="DRAM") as dp, \
             tc.tile_pool(name="work", bufs=2) as wp, \
             tc.tile_pool(name="gpsum", bufs=2, space="PSUM") as gp, \
             tc.tile_pool(name="tpsum", bufs=2, space="PSUM") as tp:

            # ---------- constants / weights to SBUF ----------
            wih_s = pp.tile([128, 3, G4], bf16)
            nc.sync.dma_start(out=wih_s[:], in_=wih_t[:].rearrange("(j p) g -> p j g", p=128))
            whh_s = pp.tile([128, 2, G4], f32r)
            nc.sync.dma_start(out=whh_s[:], in_=whh_t[:].rearrange("(j p) g -> p j g", p=128))
            bilw_s = pp.tile([128, 2, H], f32r)
            nc.sync.dma_start(out=bilw_s[:], in_=bil_w[:].rearrange("(j p) g -> p j g", p=128))
            ident_s = pp.tile([S, S], f32)
            nc.sync.dma_start(out=ident_s[:], in_=ident[:])
            bilb_s = pp.tile([RPC, 1], f32)
            nc.sync.dma_start(out=bilb_s[:], in_=bil_b[:])
            nbm_s = pp.tile([128, 8, RPC], f32)
            nc.sync.dma_start(out=nbm_s[:], in_=nbmask[:].rearrange("p (j u) -> p j u", j=8))

            cids_s = pp.tile([S, 1], i32)
            nc.sync.dma_start(out=cids_s[:], in_=cids[:])
            users_s = pp.tile([RPC, 1], i32)
            nc.sync.dma_start(out=users_s[:], in_=users[:])

            # ---------- index plumbing ----------
            tok = pp.tile([S, L], i32)
            nc.gpsimd.indirect_dma_start(
                out=tok[:], out_offset=None, in_=ctab[:],
                in_offset=bass.IndirectOffsetOnAxis(ap=cids_s[:, 0:1], axis=0))
            # bounce to DRAM in t-major order: flat[t*S + s] = tok[s, t]
            tokb = dp.tile([L * S], i32)
            nc.sync.dma_start(out=tokb[:].rearrange("(t s) -> s t", s=S), in_=tok[:])
            # reload as per-chunk offsets: toff[p, c] = flat[128*c + p]
            toff = pp.tile([128, NCHUNK], i32)
            nc.sync.dma_start(out=toff[:], in_=tokb[:].rearrange("(c p) -> p c", p=128))

            # ---------- word2vec gather (f32 -> bf16 cast) + xbar transposes ----
            # emitted lazily inside the LSTM loop (lookahead) so the scheduler
            # interleaves them with the recurrence instead of front-loading.
            gbuf = pp.tile([128, NCHUNK, EP], bf16)
            embt = pp.tile([128, 3 * NCHUNK, 128], bf16)

            def emit_chunk_fetch(cc):
                nc.gpsimd.indirect_dma_start(
                    out=gbuf[:, cc, :], out_offset=None, in_=w2v[:],
                    in_offset=bass.IndirectOffsetOnAxis(ap=toff[:, cc:cc + 1], axis=0))
                for j in range(3):
                    eng = nc.sync if (3 * cc + j) % 2 == 0 else nc.scalar
                    eng.dma_start(
                        out=embt[:, 3 * cc + j, :],
                        in_=gbuf[:, cc, 128 * j:128 * (j + 1)],
                        transpose=True)

            # ---------- LSTM recurrence ----------
            c_st = [pp.tile([S, H], f32, name="c_a"), pp.tile([S, H], f32, name="c_b")]
            nc.sync.dma_start(out=c_st[0][:], in_=zeros[:])
            # transposed hidden state, zero-padded by destination-step parity:
            # htx[p] has hT in cols [64p:64p+64], zeros elsewhere, so the rec
            # matmul writes the full 128-partition gates tile at base 0.
            htx = [pp.tile([128, 2, 128], f32r, name="htx0"),
                   pp.tile([128, 2, 128], f32r, name="htx1")]
            nc.sync.dma_start(out=htx[0][:], in_=zerosr[:].rearrange("p (k c) -> p k c", k=2))
            nc.sync.dma_start(out=htx[1][:], in_=zerosr[:].rearrange("p (k c) -> p k c", k=2))
            h_last = pp.tile([S, H], f32)

            def emit_xproj(cc, gt_tile):
                for j in range(3):
                    for n in range(2):
                        nc.tensor.matmul(
                            gt_tile[:, 512 * n:512 * (n + 1)], embt[:, 3 * cc + j, :],
                            wih_s[:, j, 512 * n:512 * (n + 1)],
                            start=(j == 0), stop=False)

            gates_tiles = {}
            for cc in range(min(1 + LOOKAHEAD, NCHUNK)):
                emit_chunk_fetch(cc)
            g0 = gp.tile([128, G4], f32, tag="gates", space="PSUM", name="gates0")
            emit_xproj(0, g0)
            gates_tiles[0] = g0

            for t in range(L):
                c = t // 2
                half = t % 2
                gates = gates_tiles[c]
                grow = gates[64 * half:64 * half + S, :] if half else gates[0:S, :]
                if t > 0:
                    for k in range(2):
                        for n in range(2):
                            nc.tensor.matmul(
                                gates[:, 512 * n:512 * (n + 1)],
                                htx[t % 2][:, k, :], whh_s[:, k, 512 * n:512 * (n + 1)],
                                start=False, stop=(k == 1 and n == 1 and half == 1))
                # cluster PE work: next chunk's xproj + fetch right after the
                # recurrent matmuls (keeps the PE in one long burst per step)
                if half == 1 and c + 1 < NCHUNK:
                    gn = gp.tile([128, G4], f32, tag="gates", space="PSUM",
                                 name=f"gates{c + 1}")
                    emit_xproj(c + 1, gn)
                    gates_tiles[c + 1] = gn
                    gates_tiles.pop(c - 1, None)
                    if c + 1 + LOOKAHEAD < NCHUNK:
                        emit_chunk_fetch(c + 1 + LOOKAHEAD)
                # activations: gate order (f, i, o, g); sigma split so the
                # f/i-dependent c-chain starts before sigma(o) completes.
                acts = wp.tile([S, 3 * H], f32, tag="acts")
                nc.scalar.activation(acts[:, 0:2 * H], grow[:, 0:2 * H],
                                     mybir.ActivationFunctionType.Sigmoid)
                gt = wp.tile([S, H], f32, tag="gt")
                nc.scalar.activation(gt[:], grow[:, 3 * H:4 * H],
                                     mybir.ActivationFunctionType.Tanh)
                nc.scalar.activation(acts[:, 2 * H:3 * H], grow[:, 2 * H:3 * H],
                                     mybir.ActivationFunctionType.Sigmoid)
                # c' = f*c + i*g
                fc = wp.tile([S, H], f32, tag="fc")
                nc.vector.tensor_tensor(out=fc[:], in0=acts[:, 0:H],
                                        in1=c_st[t % 2][:], op=mybir.AluOpType.mult)
                ig = wp.tile([S, H], f32, tag="ig")
                nc.vector.tensor_tensor(out=ig[:], in0=acts[:, H:2 * H], in1=gt[:],
                                        op=mybir.AluOpType.mult)
                c_new = c_st[(t + 1) % 2]
                nc.vector.tensor_tensor(out=c_new[:], in0=fc[:], in1=ig[:],
                                        op=mybir.AluOpType.add)
                # tanh(c), h, transpose proceed in H-halves so the first
                # recurrent matmul of step t+1 starts as early as possible
                h_t = h_last if t == L - 1 else wp.tile([S, H], f32, tag="h")
                tc_t = wp.tile([S, H], f32, tag="tc")
                np_ = (t + 1) % 2
                for k in range(2):
                    hs = slice(128 * k, 128 * (k + 1))
                    nc.scalar.activation(tc_t[:, hs], c_new[:, hs],
                                         mybir.ActivationFunctionType.Tanh)
                    nc.vector.tensor_tensor(out=h_t[:, hs], in0=acts[:, 2 * H:3 * H][:, hs],
                                            in1=tc_t[:, hs], op=mybir.AluOpType.mult)
                    htp = tp.tile([128, S], f32, tag="htp", space="PSUM")
                    nc.tensor.transpose(out=htp[:], in_=h_t[:, hs],
                                        identity=ident_s[:])
                    nc.vector.tensor_copy(htx[np_][:, k, 64 * np_:64 * np_ + S], htp[:])

            # ---------- neighbor aggregation (independent of LSTM) ----------
            u_s = pp.tile([RPC, H], f32)
            nc.gpsimd.indirect_dma_start(
                out=u_s[:], out_offset=None, in_=uemb[:],
                in_offset=bass.IndirectOffsetOnAxis(ap=users_s[:, 0:1], axis=0))
            deg_s = pp.tile([RPC, 1], f32)
            nc.gpsimd.indirect_dma_start(
                out=deg_s[:], out_offset=None, in_=deg[:],
                in_offset=bass.IndirectOffsetOnAxis(ap=users_s[:, 0:1], axis=0))
            adjv = pp.tile([RPC, K], i32)
            nc.gpsimd.indirect_dma_start(
                out=adjv[:], out_offset=None, in_=adj[:],
                in_offset=bass.IndirectOffsetOnAxis(ap=users_s[:, 0:1], axis=0))
            adjb = dp.tile([RPC * K], i32)
            nc.sync.dma_start(out=adjb[:], in_=adjv[:])
            nboff = pp.tile([128, 8], i32)
            nc.sync.dma_start(out=nboff[:], in_=adjb[:].rearrange("(j p) -> p j", p=128))
            nb = pp.tile([128, 8, H], f32)
            for j in range(8):
                nc.gpsimd.indirect_dma_start(
                    out=nb[:, j, :], out_offset=None, in_=uemb[:],
                    in_offset=bass.IndirectOffsetOnAxis(ap=nboff[:, j:j + 1], axis=0))
            nbsum_ps = tp.tile([RPC, H], f32, space="PSUM", bufs=1)
            for j in range(8):
                nc.tensor.matmul(nbsum_ps[:], nbm_s[:, j, :], nb[:, j, :],
                                 start=(j == 0), stop=(j == 7))
            degp = wp.tile([RPC, 1], f32, tag="degp")
            nc.vector.tensor_scalar_add(degp[:], deg_s[:], SMOOTH)
            rdeg = wp.tile([RPC, 1], f32, tag="rdeg")
            nc.vector.reciprocal(rdeg[:], degp[:])
            neigh = wp.tile([RPC, H], f32, tag="neigh")
            nc.vector.tensor_scalar_mul(neigh[:], nbsum_ps[:], rdeg[:, 0:1])
            diff = pp.tile([RPC, H], f32)
            nc.vector.tensor_tensor(out=diff[:], in0=u_s[:], in1=neigh[:],
                                    op=mybir.AluOpType.subtract)
            dn2 = wp.tile([RPC, 1], f32, tag="dn2")
            djunk = wp.tile([RPC, H], f32, tag="djunk")
            nc.scalar.activation(djunk[:], diff[:],
                                 mybir.ActivationFunctionType.Square,
                                 accum_out=dn2[:])
            dnorm = wp.tile([RPC, 1], f32, tag="dnorm")
            nc.scalar.sqrt(dnorm[:], dn2[:])
            dnormc = wp.tile([RPC, 1], f32, tag="dnormc")
            nc.vector.tensor_scalar_max(dnormc[:], dnorm[:], 1e-12)
            rdn = wp.tile([RPC, 1], f32, tag="rdn")
            nc.vector.reciprocal(rdn[:], dnormc[:])
            reg_t = pp.tile([RPC, H], f32)
            nc.vector.tensor_scalar_mul(reg_t[:], diff[:], rdn[:, 0:1])
            nc.sync.dma_start(out=out_reg[:], in_=reg_t[:])

            # ---------- scoring head ----------
            bil_ps = tp.tile([RPC, H], f32, tag="bil", space="PSUM", bufs=1)
            for k in range(2):
                nc.tensor.matmul(bil_ps[:], htx[0][:, k, 0:RPC], bilw_s[:, k, :],
                                 start=(k == 0), stop=(k == 1))
            qh = h_last[0:RPC, :]
            ah = h_last[RPC:S, :]
            mjunk = wp.tile([RPC, H], f32, tag="mjunk")
            match = wp.tile([RPC, 1], f32, tag="match")
            mprod = wp.tile([RPC, H], f32, tag="mprod")
            nc.vector.tensor_tensor(out=mprod[:], in0=bil_ps[:], in1=ah,
                                    op=mybir.AluOpType.mult)
            nc.scalar.activation(mjunk[:], mprod[:],
                                 mybir.ActivationFunctionType.Copy,
                                 accum_out=match[:])
            # cosine(qh, u)
            qu = wp.tile([RPC, 1], f32, tag="qu")
            qjunk = wp.tile([RPC, H], f32, tag="qjunk")
            qprod = wp.tile([RPC, H], f32, tag="qprod")
            nc.vector.tensor_tensor(out=qprod[:], in0=qh, in1=u_s[:],
                                    op=mybir.AluOpType.mult)
            nc.scalar.activation(qjunk[:], qprod[:],
                                 mybir.ActivationFunctionType.Copy,
                                 accum_out=qu[:])
            nrm2 = wp.tile([RPC, 2], f32, tag="nrm2")
            nc.scalar.activation(qjunk[:], qh,
                                 mybir.ActivationFunctionType.Square,
                                 accum_out=nrm2[:, 0:1])
            nc.scalar.activation(qjunk[:], u_s[:],
                                 mybir.ActivationFunctionType.Square,
                                 accum_out=nrm2[:, 1:2])
            nrm = wp.tile([RPC, 2], f32, tag="nrm")
            nc.scalar.sqrt(nrm[:], nrm2[:])
            nrmc = wp.tile([RPC, 2], f32, tag="nrmc")
            nc.vector.tensor_scalar_max(nrmc[:], nrm[:], 1e-8)
            nprod = wp.tile([RPC, 1], f32, tag="nprod")
            nc.vector.tensor_tensor(out=nprod[:], in0=nrmc[:, 0:1], in1=nrmc[:, 1:2],
                                    op=mybir.AluOpType.mult)
            rn = wp.tile([RPC, 1], f32, tag="rn")
            nc.vector.reciprocal(rn[:], nprod[:])
            cosv = wp.tile([RPC, 1], f32, tag="cosv")
            nc.vector.tensor_tensor(out=cosv[:], in0=qu[:], in1=rn[:],
                                    op=mybir.AluOpType.mult)
            mb_t = wp.tile([RPC, 1], f32, tag="mb")
            nc.vector.tensor_tensor(out=mb_t[:], in0=match[:], in1=bilb_s[:],
                                    op=mybir.AluOpType.add)
            res_t = wp.tile([RPC, 1], f32, tag="res")
            nc.vector.tensor_tensor(out=res_t[:], in0=mb_t[:], in1=cosv[:],
                                    op=mybir.AluOpType.mult)
            nc.sync.dma_start(out=out_res[:], in_=res_t[:])

    from tile_patch import split_excess_waits
    split_excess_waits(nc)
    nc.compile()
    return nc


def _host_prep(inputs):
    q = np.asarray(inputs["question_list"]).astype(np.int64)
    a = np.asarray(inputs["answer_list"]).astype(np.int64)
    ul = np.asarray(inputs["user_list"]).astype(np.int64)
    word2vec = np.asarray(inputs["word2vec"], dtype=np.float32)
    ctab = np.asarray(inputs["content_table"]).astype(np.int32)
    uemb = np.asarray(inputs["user_embed_table"], dtype=np.float32)
    adj = np.asarray(inputs["adj"]).astype(np.int32)
    deg = np.asarray(inputs["degree"], dtype=np.float32)
    Wih = np.asarray(inputs["Wih"], dtype=np.float32)
    Whh = np.asarray(inputs["Whh"], dtype=np.float32)
    b = np.asarray(inputs["b"], dtype=np.float32)
    bil_W = np.asarray(inputs["bil_W"], dtype=np.float32)
    bil_b = np.asarray(inputs["bil_b"], dtype=np.float32)

    # gate permutation (i, f, g, o) -> (f, i, o, g)
    perm = np.concatenate([np.arange(H, 2 * H), np.arange(0, H),
                           np.arange(3 * H, 4 * H), np.arange(2 * H, 3 * H)])
    Wih_p, Whh_p, b_p = Wih[perm], Whh[perm], b[perm]

    w2v_pad = np.zeros((V, EP), np.float32)
    w2v_pad[:, :E] = word2vec
    w2v_pad[:, EP - 1] = 1.0                      # ones column -> bias row
    wih_t = np.zeros((EP, G4), np.float32)
    wih_t[:E, :] = Wih_p.T
    wih_t[EP - 1, :] = b_p
    wih_t = wih_t.astype(ml_dtypes.bfloat16)
    whh_t = np.ascontiguousarray(Whh_p.T)
    bil_w = np.ascontiguousarray(bil_W[0])

    # neighbor-sum masks: mask[p, j, u] = 1 iff 4*j + p//32 == u
    nbm = np.zeros((128, 8, RPC), np.float32)
    for j in range(8):
        for grp in range(4):
            nbm[32 * grp:32 * (grp + 1), j, 4 * j + grp] = 1.0
    nbm = nbm.reshape(128, 8 * RPC)

    shared = {
        "w2v": w2v_pad, "ctab": ctab, "wih_t": wih_t, "whh_t": whh_t,
        "bil_w": bil_w, "bil_b": np.full((RPC, 1), float(bil_b[0]), np.float32),
        "uemb": uemb, "adj": adj, "deg": deg, "nbmask": nbm,
        "ident": np.eye(S, dtype=np.float32),
        "zeros": np.zeros((S, H), np.float32),
        "zerosr": np.zeros((128, H), np.float32),
    }
    in_maps = []
    for i in range(N_CORES):
        rows = slice(RPC * i, RPC * (i + 1))
        cids = np.concatenate([q[rows], a[rows]]).astype(np.int32) - U
        m = dict(shared)
        m["cids"] = cids.reshape(S, 1)
        m["users"] = ul[rows].astype(np.int32).reshape(RPC, 1)
        in_maps.append(m)
    return in_maps


def kernel(**inputs):
    from concourse.bass_utils import run_bass_kernel_spmd
    if "nc" not in _CACHE:
        _CACHE["nc"] = _build()
    nc = _CACHE["nc"]
    in_maps = _host_prep(inputs)
    out = run_bass_kernel_spmd(nc, in_maps, core_ids=list(range(N_CORES)))
    result = np.concatenate([out.results[i]["res"] for i in range(N_CORES)], axis=0)
    regular = np.concatenate([out.results[i]["reg"] for i in range(N_CORES)], axis=0)
    return result, regular
